# revision 34
# baseline (speedup 1.0000x reference)
"""Trainium2 Bass kernel for the ADMM total-variation solver (nn_ADMM).

Math: x <- B^-1(bA + v) iterated 50x, B = AtA + g*DtD + a*I.  AtA is
rank-9 and C := g*DtD + a*I is circulant, so by Woodbury
    B^-1 = C^-1 - W2 U^T,   U = C^-1 A^T,  W2 = U S^-1,  S = I9 + A U.
C^-1 is applied as a banded (radius-32) circular convolution G; the
rank-9 correction uses q = U^T v.  All 8 cores run the same program
(SPMD, no collectives); core 0's output is returned.

v5 design - the rank-10 correction is applied entirely on the PE via
diagonal-masked q tiles, so nothing downstream waits for an explicit
x = E - corr materialization:
- q partials: Z1 = U2 (.) v (DVE mul), grouped c-reduce (DVE) -> Z1r
  [128, 12] (col 9 = -1/128 so q9 = -1 rides the c0 column; 10,11 = 0).
- Qrep: 3 matmuls, lhsT = Z1r cols [4g:4g+4] read through a stride-0
  broadcast AP [128,(32x0),(4x1)] and rhs = ones: one [128, 96] PSUM
  bank holds q_{4g+mw} at partition p = 4c'+mw, replicated.  This IS
  the partition reduce - no separate ones-matmul or q round-trip.
- qS = M96 (.) Qrep (one DVE mul): qS[4c'+mw, 32g+c] = q_{4g+mw} d_{c'c}.
- Every rank-10 operator application M W2e q is then ONE matmul per g
  with a precomputed structured lhsT: -(AB W2e) q into the next bankAt,
  -(B W2e) q into the next bankB, +(a W2e) q into the next bankCx
  (= -a*x), and -W2e q into bankE on the last iteration only - x never
  materializes in SBUF until the final output.  The next iteration's
  banded bursts run on Esb = bf16(E), cast mid-tail, long before the
  correction resolves.
- state algebra: tAB := eta - Bx (f32, DVE-seeded into the next two
  PSUM banks; PE accumulates on top), tTW := tau - a*x:
  tAB' = bankB - Un,  u3 = tTW - bankCx (= tau),
  tTW' = u3 - relu(u3 - bankCx) = min(u3, bankCx),  v = -tTW' - tAB'.

Vector layout: [128, 32] tiles, flat index i = k + 128*c at tile col c.
"""

import numpy as np

N = 4096
P = 128          # partitions
CCOL = 32        # payload columns; i = k + 128*c
RB = 32          # band radius of G
R9 = 9           # Woodbury rank
R12 = 12         # padded rank columns (9 real + c0 + 2 zero)
NG = 3           # mask groups of 4
GAMMA = 10.0
ALPHA = 5.0
LAM = 1e-4
NIT = 50
NCORES = 8

# f32-column offsets inside the constant blob [128, BLOB_COLS].
# bf16 payloads are packed two-per-f32-column and bitcast on device.
_cur = 0
def _alloc(w):
    global _cur
    off = _cur
    _cur += w
    return off

OFF_A1M   = _alloc(64)    # gamma*(S+ - I) main, bf16 [128,128] (iter 1)
OFF_BM    = _alloc(64)    # gamma*(S- - I) main
OFF_ABM   = _alloc(64)    # gamma*(S+ + S- - 2I) main (A1_M + B_M)
OFF_A1C   = _alloc(64)    # A1 corner (single element, padded)
OFF_BC    = _alloc(64)    # B corner
OFF_GHIM  = _alloc(64)    # C^-1 banded main, hi half
OFF_GLOM  = _alloc(64)    # lo half
OFF_GBLH  = _alloc(64)    # C^-1 left corner hi
OFF_GBHH  = _alloc(64)    # C^-1 right corner hi
OFF_ONES  = _alloc(64)    # all-ones bf16 (Qrep rhs)
OFF_NAI   = _alloc(64)    # -alpha * I bf16 (bankCx plain part)
OFF_U2    = _alloc(144)   # U m-major: [k, m*32+c], bf16 [128, 288]
OFF_X0    = _alloc(16)    # x0 tile, bf16 [128, 32]
OFF_M96   = _alloc(48)    # qS mask [128, 96]: d_{p//4, col%32}
OFF_NIB   = _alloc(64)    # -I bf16 (state seed via hi/lo bf16 matmuls)
OFF_WS    = _alloc(64 * NG)    # -W2e blocks      -> bankE (last iter)
OFF_ABS   = _alloc(64 * NG)    # -(AB W2e) blocks -> next bankAt
OFF_BS    = _alloc(64 * NG)    # -(B W2e) blocks  -> next bankB
OFF_CS    = _alloc(64 * NG)    # +(a W2e) blocks  -> next bankCx
BLOB_COLS = _cur


def _bf16(x):
    x32 = np.asarray(x, np.float32)
    u = x32.view(np.uint32)
    r = ((u >> 16) + ((u >> 15) & 1)).astype(np.uint32) << 16
    return r.view(np.float32)


def _pack_bf16(arr):
    """[128, W] float (W even) -> [128, W//2] f32 with packed bf16 pairs."""
    a = _bf16(arr).view(np.uint32) >> 16
    lo, hi = a[:, 0::2], a[:, 1::2]
    return (lo | (hi << 16)).view(np.float32)


def _banded(h):
    """main/BL/BH lhsT pieces for kernel h (dict d -> coef), [128,128] each.
    lhsT[k, m]: contraction index k = input row, m = output row.
    main: within-column (shift d = k - m);
    BL: rhs = col c-1 view (shift d = k - 128 - m);
    BH: rhs = col c+1 view (shift d = k + 128 - m)."""
    B0 = np.zeros((P, P)); BL = np.zeros((P, P)); BH = np.zeros((P, P))
    for k in range(P):
        for m in range(P):
            if (k - m) in h:
                B0[k, m] = h[k - m]
            if (k - P - m) in h:
                BL[k, m] = h[k - P - m]
            if (k + P - m) in h:
                BH[k, m] = h[k + P - m]
    return B0, BL, BH


def _tile(vec):
    """[4096] -> [128, 32], i = k + 128*c."""
    return np.ascontiguousarray(np.asarray(vec).reshape(CCOL, P).T)


def _mask_blocks(M12):
    """[4096, 12] operator matrix -> NG lhsT blocks [128, 128]:
    blk_g[p = 4*c' + mw, k] = M12[k + 128*c', 4g + mw]."""
    T = M12.reshape(CCOL, P, R12)        # [c', k, m]
    out = []
    for g in range(NG):
        blk = np.zeros((P, P))
        for cp in range(CCOL):
            for mw in range(4):
                blk[4 * cp + mw, :] = T[cp, :, 4 * g + mw]
        out.append(blk)
    return out


def host_constants(target, A, x0):
    """All f64 precompute; returns the [128, BLOB_COLS] f32 device blob."""
    A64 = np.asarray(A, np.float64)
    w = ALPHA + 2 * GAMMA * (1 - np.cos(2 * np.pi * np.arange(N // 2 + 1) / N))

    def C_inv(z):
        return np.fft.irfft(np.fft.rfft(z, axis=-1) / w, n=N, axis=-1)

    U = C_inv(A64).T                              # [N, 9]
    S = np.eye(R9) + A64 @ U
    W2 = U @ np.linalg.inv(S)                     # [N, 9]
    g = np.fft.irfft(1.0 / w, n=N)                # kernel of C^-1
    b = A64 @ np.asarray(target, np.float64)
    bA = b @ A64
    c0 = C_inv(bA) - W2 @ (U.T @ bA)              # B^-1 @ bA

    blob = np.zeros((P, BLOB_COLS), np.float32)

    def putb(off, arr):
        p = _pack_bf16(arr)
        blob[:p.shape[0], off:off + p.shape[1]] = p

    mA1 = _banded({-1: GAMMA, 0: -GAMMA})         # gamma*(S+ - I)
    mB = _banded({1: GAMMA, 0: -GAMMA})           # gamma*(S- - I)
    mG = _banded({d: g[d % N] for d in range(-RB, RB + 1)})
    Ghi = [_bf16(m).astype(np.float64) for m in mG]
    Glo = [m - h for m, h in zip(mG, Ghi)]

    putb(OFF_A1M, mA1[0]); putb(OFF_A1C, mA1[1])
    putb(OFF_BM, mB[0]); putb(OFF_BC, mB[2])
    putb(OFF_ABM, mA1[0] + mB[0])
    putb(OFF_GHIM, Ghi[0]); putb(OFF_GLOM, Glo[0])
    putb(OFF_GBLH, Ghi[1])
    putb(OFF_GBHH, Ghi[2])
    putb(OFF_ONES, np.ones((P, P)))
    putb(OFF_NAI, -ALPHA * np.eye(P))

    # U2[k, m*32+c] = U[k + 128c, m] (m-major)
    putb(OFF_U2, U.reshape(CCOL, P, R9).transpose(1, 2, 0).reshape(P, R9 * CCOL))
    putb(OFF_X0, _tile(np.asarray(x0, np.float64)))

    # qS mask: M96[p, 32g + c] = (p//4 == c)
    m96 = np.zeros((P, NG * CCOL))
    for p in range(P):
        m96[p, (p // 4)::CCOL] = 1.0
    putb(OFF_M96, m96)
    putb(OFF_NIB, -np.eye(P))

    # W2e padded to 12 columns: [W2 | c0 | 0 | 0]
    W2e = np.concatenate([W2, c0[:, None], np.zeros((N, 2))], axis=1)
    AB_W2e = GAMMA * (np.roll(W2e, 1, axis=0) + np.roll(W2e, -1, axis=0)
                      - 2 * W2e)
    B_W2e = GAMMA * (np.roll(W2e, -1, axis=0) - W2e)

    for gi, blk in enumerate(_mask_blocks(-W2e)):
        putb(OFF_WS + 64 * gi, blk)
    for gi, blk in enumerate(_mask_blocks(-AB_W2e)):
        putb(OFF_ABS + 64 * gi, blk)
    for gi, blk in enumerate(_mask_blocks(-B_W2e)):
        putb(OFF_BS + 64 * gi, blk)
    for gi, blk in enumerate(_mask_blocks(ALPHA * W2e)):
        putb(OFF_CS + 64 * gi, blk)
    return np.ascontiguousarray(blob)


def build_nc():
    """Build and compile the Bacc graph (one core's program)."""
    from concourse import bacc, mybir, tile

    f32 = mybir.dt.float32
    bf16 = mybir.dt.bfloat16
    Alu = mybir.AluOpType
    nc = bacc.Bacc(target_bir_lowering=False)

    blob_ext = nc.declare_dram_parameter("blob", [P, BLOB_COLS], f32, isOutput=False)
    out_ext = nc.declare_dram_parameter("out", [P, CCOL], f32, isOutput=True)

    with tile.TileContext(nc) as tc:
        with (
            tc.tile_pool(name="const", bufs=1) as cpool,
            tc.tile_pool(name="work", bufs=3) as wpool,
            tc.tile_pool(name="psum", bufs=1, space="PSUM") as ppool,
        ):
            cb = cpool.tile([P, BLOB_COLS], f32, tag="blob")
            nc.sync.dma_start(cb[:, :], blob_ext[:, :])

            def csb(off, wcols):
                """bf16 view of wcols f32 columns -> [128, 2*wcols] bf16"""
                return cb[:, off:off + wcols].bitcast(bf16)

            A1_M, A1_C = csb(OFF_A1M, 64), csb(OFF_A1C, 64)
            B_M, B_C = csb(OFF_BM, 64), csb(OFF_BC, 64)
            AB_M = csb(OFF_ABM, 64)
            GHI_M, GLO_M = csb(OFF_GHIM, 64), csb(OFF_GLOM, 64)
            GBL_H = csb(OFF_GBLH, 64)
            GBH_H = csb(OFF_GBHH, 64)
            ones_bf = csb(OFF_ONES, 64)
            NAI = csb(OFF_NAI, 64)
            U2 = csb(OFF_U2, 144)                  # [128, 288] bf16
            U2_3d = U2.rearrange("k (m c) -> k m c", c=CCOL)
            M96 = csb(OFF_M96, 48)                 # [128, 96] bf16
            NIB = csb(OFF_NIB, 64)                 # [128, 128] bf16 -I
            WS = [csb(OFF_WS + 64 * g, 64) for g in range(NG)]
            ABS = [csb(OFF_ABS + 64 * g, 64) for g in range(NG)]
            BS = [csb(OFF_BS + 64 * g, 64) for g in range(NG)]
            CS = [csb(OFF_CS + 64 * g, 64) for g in range(NG)]

            def corner(bank, lhsT, src, shift, stop=False, start=False):
                """Cross-column corner of a banded circulant: out col c
                reads src col c+shift (mod 32), as two payload matmuls."""
                if shift == -1:
                    nc.tensor.matmul(bank[:, 1:CCOL], lhsT, src[:, 0:CCOL - 1],
                                     start=start, stop=False,
                                     skip_group_check=True)
                    nc.tensor.matmul(bank[:, 0:1], lhsT, src[:, CCOL - 1:CCOL],
                                     start=start, stop=stop,
                                     skip_group_check=True)
                else:
                    nc.tensor.matmul(bank[:, 0:CCOL - 1], lhsT, src[:, 1:CCOL],
                                     start=start, stop=False,
                                     skip_group_check=True)
                    nc.tensor.matmul(bank[:, CCOL - 1:CCOL], lhsT, src[:, 0:1],
                                     start=start, stop=stop,
                                     skip_group_check=True)

            # persistent tiles (Z1r double-buffered to relax the WAR between
            # one iteration's reduce and the previous Qrep weight loads)
            Z1rA = cpool.tile([P, 16], bf16, tag="Z1rA")
            Z1rB = cpool.tile([P, 16], bf16, tag="Z1rB")
            for zz in (Z1rA, Z1rB):
                nc.vector.memset(zz[:, :], 0.0)
                nc.vector.memset(zz[:, R9:R9 + 1], -1.0 / P)
            bankQ = ppool.tile([P, NG * CCOL], f32, tag="Q")

            def at_bank(j):
                return ppool.tile([P, CCOL], f32, tag=f"At{j % 2}",
                                  name=f"At{j % 2}")
            def b_bank(j):
                return ppool.tile([P, CCOL], f32, tag=f"B{j % 2}",
                                  name=f"B{j % 2}")
            def cx_bank(j):
                return ppool.tile([P, CCOL], f32, tag=f"Cx{j % 2}",
                                  name=f"Cx{j % 2}")

            # --- prologue: iteration-1 banks straight from x0 ---
            x0m = csb(OFF_X0, 16)[:, 0:CCOL]
            bankCx = cx_bank(1)
            nc.tensor.matmul(bankCx[:, :], NAI, x0m, start=True, stop=True)
            bankAt = at_bank(1)
            nc.tensor.matmul(bankAt[:, :], A1_M, x0m, start=True, stop=False)
            corner(bankAt, A1_C, x0m, -1, stop=True)
            bankB = None
            T3 = None

            for j in range(1, NIT + 1):
                first = (j == 1)
                last = (j == NIT)

                # --- DVE: tau chain ---
                # u3 = tTW - (-a*x) = tau;  tTW' = min(u3, -a*x)
                t3n = wpool.tile([P, CCOL], f32, tag=f"t3{j % 2}")
                if first:
                    nc.vector.tensor_scalar_min(t3n[:, :], bankCx[:, :], 0.0)
                else:
                    u3t = wpool.tile([P, CCOL], f32, tag="u3")
                    nc.vector.tensor_sub(u3t[:, :], T3[:, :], bankCx[:, :])
                    nc.vector.tensor_tensor(t3n[:, :], u3t[:, :], bankCx[:, :],
                                            Alu.min)

                # --- DVE: soft-threshold and v (t2 = Un - eta = -tAB') ---
                r1 = wpool.tile([P, CCOL], f32, tag="r1")
                Un = wpool.tile([P, CCOL], f32, tag="Un")
                vh = wpool.tile([P, CCOL], bf16, tag="vh")
                vm = vh[:, 0:CCOL]
                nc.vector.tensor_scalar(r1[:, :], bankAt[:, :], -LAM, LAM,
                                        Alu.max, Alu.min)
                nc.vector.tensor_sub(Un[:, :], bankAt[:, :], r1[:, :])
                if first:
                    t2 = Un
                    nc.vector.tensor_sub(vm, Un[:, :], t3n[:, :])
                else:
                    t2 = wpool.tile([P, CCOL], f32, tag="t2")
                    nc.vector.tensor_sub(t2[:, :], Un[:, :], bankB[:, :])
                    nc.vector.tensor_sub(vm, t2[:, :], t3n[:, :])
                if not last:
                    # hi/lo bf16 split of the state seed t2 (PE applies
                    # -I @ (t2h + t2l) into the next At/B banks).  Priority-
                    # pushed late so they fill the Qrep wait, not the q-path.
                    t2h = wpool.tile([P, CCOL], bf16, tag="t2h")
                    t2l = wpool.tile([P, CCOL], bf16, tag="t2l")
                    with tc.high_priority(offset=-14):
                        nc.vector.tensor_copy(t2h[:, :], t2[:, :])
                        nc.vector.scalar_tensor_tensor(t2l[:, :], t2[:, :],
                                                       1.0, t2h[:, :],
                                                       Alu.mult, Alu.subtract)

                # --- PE: banded G apply on v ---
                bankE = ppool.tile([P, CCOL], f32, tag="E")
                nc.tensor.matmul(bankE[:, :], GHI_M, vm, start=True, stop=False)
                nc.tensor.matmul(bankE[:, :], GLO_M, vm, start=False, stop=False)
                corner(bankE, GBL_H, vm, -1)
                corner(bankE, GBH_H, vm, +1, stop=(not last))

                # --- DVE: rank-9 head  Z1 = U2 (.) v ; grouped c-reduce ---
                Z1r = Z1rA if j % 2 else Z1rB
                Z1 = wpool.tile([P, R9 * CCOL], bf16, tag="Z1")
                z1_3d = Z1[:, :].rearrange("k (m c) -> k m c", c=CCOL)
                vb9 = vm.unsqueeze(1).broadcast_to([P, R9, CCOL])
                nc.vector.tensor_mul(z1_3d, U2_3d, vb9)
                with nc.allow_low_precision(reason="q partials consumed f32"):
                    nc.vector.tensor_reduce(Z1r[:, 0:R9], z1_3d,
                                            axis=mybir.AxisListType.X,
                                            op=Alu.add)

                # --- DVE: replicate Z1r into matmul-ready lhsT layout ---
                # Z1rep[k, 128g + 4c' + mw] = Z1r[k, 4g + mw]
                Z1rep = wpool.tile([P, NG * P], bf16, tag="Z1rep")
                zin = Z1r[:, 0:R12].rearrange("k (g m) -> k g m", m=4) \
                    .unsqueeze(2).broadcast_to([P, NG, CCOL, 4])
                zout = Z1rep[:, :].rearrange("k (g c m) -> k g c m",
                                             c=CCOL, m=4)
                nc.vector.tensor_copy(zout, zin)

                # --- PE: pipeline-warming fillers (gated on Z1rep so they
                #     run back-to-back right before the Qrep matmuls) ---
                for _f in range(2):
                    nc.tensor.matmul(bankQ[0:1, 0:1], Z1rep[:, 0:1],
                                     ones_bf[:, 0:1], start=True, stop=True,
                                     skip_group_check=True)

                # --- PE: Qrep (partition reduce + replicate of q) ---
                for g in range(NG):
                    nc.tensor.matmul(bankQ[:, CCOL * g:CCOL * (g + 1)],
                                     Z1rep[:, P * g:P * (g + 1)],
                                     ones_bf[:, 0:CCOL],
                                     start=True, stop=True,
                                     skip_group_check=True)

                # --- DVE: qS = mask (.) Qrep; then Esb cast (priority-pushed
                #     later so the scheduler cannot hoist it into the q-path)
                qS = wpool.tile([P, NG * CCOL], bf16, tag="qS")
                nc.vector.tensor_mul(qS[:, :], M96[:, 0:NG * CCOL],
                                     bankQ[:, :])
                if not last:
                    Enew = wpool.tile([P, CCOL], bf16, tag="Esb")
                    with tc.high_priority(offset=-24):
                        nc.vector.tensor_copy(Enew[:, :], bankE[:, :])

                # --- PE: masked rank-10 parts + plain banded bursts that
                #     COMPLETE the next iteration's banks ---
                if last:
                    for g in range(NG):
                        nc.tensor.matmul(bankE[:, :], WS[g],
                                         qS[:, CCOL * g:CCOL * (g + 1)],
                                         start=False, stop=(g == NG - 1),
                                         skip_group_check=True)
                else:
                    at_n = at_bank(j + 1)
                    b_n = b_bank(j + 1)
                    cx_n = cx_bank(j + 1)
                    em = Enew[:, 0:CCOL]
                    # bankCx_{j+1} first: the next tau chain (u3/min) can
                    # then run in the DVE idle window before r1
                    for g in range(NG):
                        nc.tensor.matmul(cx_n[:, :], CS[g],
                                         qS[:, CCOL * g:CCOL * (g + 1)],
                                         start=(g == 0), stop=False,
                                         skip_group_check=True)
                    nc.tensor.matmul(cx_n[:, :], NAI, em, start=False,
                                     stop=True, skip_group_check=True)
                    # bankAt_{j+1}: bf16 hi/lo state seed, masked, plain
                    nc.tensor.matmul(at_n[:, :], NIB, t2h[:, :],
                                     start=True, stop=False,
                                     skip_group_check=True)
                    nc.tensor.matmul(at_n[:, :], NIB, t2l[:, :],
                                     start=False, stop=False,
                                     skip_group_check=True)
                    for g in range(NG):
                        nc.tensor.matmul(at_n[:, :], ABS[g],
                                         qS[:, CCOL * g:CCOL * (g + 1)],
                                         start=False, stop=False,
                                         skip_group_check=True)
                    nc.tensor.matmul(at_n[:, :], AB_M, em, start=False,
                                     stop=False, skip_group_check=True)
                    corner(at_n, A1_C, em, -1)
                    corner(at_n, B_C, em, +1, stop=True)
                    # bankB_{j+1}
                    nc.tensor.matmul(b_n[:, :], NIB, t2h[:, :],
                                     start=True, stop=False,
                                     skip_group_check=True)
                    nc.tensor.matmul(b_n[:, :], NIB, t2l[:, :],
                                     start=False, stop=False,
                                     skip_group_check=True)
                    for g in range(NG):
                        nc.tensor.matmul(b_n[:, :], BS[g],
                                         qS[:, CCOL * g:CCOL * (g + 1)],
                                         start=False, stop=False,
                                         skip_group_check=True)
                    nc.tensor.matmul(b_n[:, :], B_M, em, start=False,
                                     stop=False, skip_group_check=True)
                    corner(b_n, B_C, em, +1, stop=True)

                if not last:
                    bankAt, bankB, bankCx, T3 = at_n, b_n, cx_n, t3n
                else:
                    Xout = wpool.tile([P, CCOL], f32, tag="Xout")
                    nc.vector.tensor_copy(Xout[:, :], bankE[:, :])
                    nc.sync.dma_start(out_ext[:, :], Xout[:, :])

    nc.compile()
    return nc


def kernel(**inputs):
    from concourse.bass_utils import run_bass_kernel_spmd

    target = np.asarray(inputs["target"], np.float32)
    A = np.asarray(inputs["A"], np.float32)
    x0 = np.asarray(inputs["x0"], np.float32)

    blob = host_constants(target, A, x0)
    nc = build_nc()
    in_maps = [{"blob": blob} for _ in range(NCORES)]
    res = run_bass_kernel_spmd(nc, in_maps, core_ids=list(range(NCORES)))
    out_tile = np.asarray(res.results[0]["out"], np.float32)
    return np.ascontiguousarray(out_tile.T.reshape(-1))


# revision 39
# speedup vs baseline: 1.1117x; 1.1117x over previous
"""Trainium2 Bass kernel for the ADMM total-variation solver (nn_ADMM).

Math: x <- B^-1(bA + v) iterated 50x, B = AtA + g*DtD + a*I.  AtA is
rank-9 and C := g*DtD + a*I is circulant, so by Woodbury
    B^-1 = C^-1 - W2 U^T,   U = C^-1 A^T,  W2 = U S^-1,  S = I9 + A U.
C^-1 is applied as a banded (radius-32) circular convolution G; the
rank-9 correction uses q = U^T v.  All 8 cores run the same program
(SPMD, no collectives); core 0's output is returned.

v5 design - the rank-10 correction is applied entirely on the PE via
diagonal-masked q tiles, so nothing downstream waits for an explicit
x = E - corr materialization:
- q partials: Z1 = U2 (.) v (DVE mul), grouped c-reduce (DVE) -> Z1r
  [128, 12] (col 9 = -1/128 so q9 = -1 rides the c0 column; 10,11 = 0).
- Qrep: 3 matmuls, lhsT = Z1r cols [4g:4g+4] read through a stride-0
  broadcast AP [128,(32x0),(4x1)] and rhs = ones: one [128, 96] PSUM
  bank holds q_{4g+mw} at partition p = 4c'+mw, replicated.  This IS
  the partition reduce - no separate ones-matmul or q round-trip.
- qS = M96 (.) Qrep (one DVE mul): qS[4c'+mw, 32g+c] = q_{4g+mw} d_{c'c}.
- Every rank-10 operator application M W2e q is then ONE matmul per g
  with a precomputed structured lhsT: -(AB W2e) q into the next bankAt,
  -(B W2e) q into the next bankB, +(a W2e) q into the next bankCx
  (= -a*x), and -W2e q into bankE on the last iteration only - x never
  materializes in SBUF until the final output.  The next iteration's
  banded bursts run on Esb = bf16(E), cast mid-tail, long before the
  correction resolves.
- state algebra: tAB := eta - Bx (f32, DVE-seeded into the next two
  PSUM banks; PE accumulates on top), tTW := tau - a*x:
  tAB' = bankB - Un,  u3 = tTW - bankCx (= tau),
  tTW' = u3 - relu(u3 - bankCx) = min(u3, bankCx),  v = -tTW' - tAB'.

Vector layout: [128, 32] tiles, flat index i = k + 128*c at tile col c.
"""

import numpy as np

N = 4096
P = 128          # partitions
CCOL = 32        # payload columns; i = k + 128*c
RB = 32          # band radius of G
R9 = 9           # Woodbury rank
R12 = 12         # padded rank columns (9 real + c0 + 2 zero)
NG = 3           # mask groups of 4
GAMMA = 10.0
ALPHA = 5.0
LAM = 1e-4
NIT = 50
NCORES = 8

# f32-column offsets inside the constant blob [128, BLOB_COLS].
# bf16 payloads are packed two-per-f32-column and bitcast on device.
_cur = 0
def _alloc(w):
    global _cur
    off = _cur
    _cur += w
    return off

OFF_A1M   = _alloc(64)    # gamma*(S+ - I) main, bf16 [128,128] (iter 1)
OFF_BM    = _alloc(64)    # gamma*(S- - I) main
OFF_ABM   = _alloc(64)    # gamma*(S+ + S- - 2I) main (A1_M + B_M)
OFF_A1C   = _alloc(64)    # A1 corner (single element, padded)
OFF_BC    = _alloc(64)    # B corner
OFF_GHIM  = _alloc(64)    # C^-1 banded main, hi half
OFF_GLOM  = _alloc(64)    # lo half
OFF_GBLH  = _alloc(64)    # C^-1 left corner hi
OFF_GBHH  = _alloc(64)    # C^-1 right corner hi
OFF_ONES  = _alloc(64)    # all-ones bf16 (Qrep rhs)
OFF_NAI   = _alloc(64)    # -alpha * I bf16 (bankCx plain part)
OFF_U2    = _alloc(144)   # U m-major: [k, m*32+c], bf16 [128, 288]
OFF_X0    = _alloc(16)    # x0 tile, bf16 [128, 32]
OFF_M96   = _alloc(48)    # qS mask [128, 96]: d_{p//4, col%32}
OFF_NIB   = _alloc(64)    # -I bf16 (state seed via hi/lo bf16 matmuls)
OFF_WS    = _alloc(64 * NG)    # -W2e blocks      -> bankE (last iter)
OFF_ABS   = _alloc(64 * NG)    # -(AB W2e) blocks -> next bankAt
OFF_BS    = _alloc(64 * NG)    # -(B W2e) blocks  -> next bankB
OFF_CS    = _alloc(64 * NG)    # +(a W2e) blocks  -> next bankCx
BLOB_COLS = _cur


def _bf16(x):
    x32 = np.asarray(x, np.float32)
    u = x32.view(np.uint32)
    r = ((u >> 16) + ((u >> 15) & 1)).astype(np.uint32) << 16
    return r.view(np.float32)


def _pack_bf16(arr):
    """[128, W] float (W even) -> [128, W//2] f32 with packed bf16 pairs."""
    a = _bf16(arr).view(np.uint32) >> 16
    lo, hi = a[:, 0::2], a[:, 1::2]
    return (lo | (hi << 16)).view(np.float32)


def _banded(h):
    """main/BL/BH lhsT pieces for kernel h (dict d -> coef), [128,128] each.
    lhsT[k, m]: contraction index k = input row, m = output row.
    main: within-column (shift d = k - m);
    BL: rhs = col c-1 view (shift d = k - 128 - m);
    BH: rhs = col c+1 view (shift d = k + 128 - m)."""
    B0 = np.zeros((P, P)); BL = np.zeros((P, P)); BH = np.zeros((P, P))
    for k in range(P):
        for m in range(P):
            if (k - m) in h:
                B0[k, m] = h[k - m]
            if (k - P - m) in h:
                BL[k, m] = h[k - P - m]
            if (k + P - m) in h:
                BH[k, m] = h[k + P - m]
    return B0, BL, BH


def _tile(vec):
    """[4096] -> [128, 32], i = k + 128*c."""
    return np.ascontiguousarray(np.asarray(vec).reshape(CCOL, P).T)


def _mask_blocks(M12):
    """[4096, 12] operator matrix -> NG lhsT blocks [128, 128]:
    blk_g[p = 4*c' + mw, k] = M12[k + 128*c', 4g + mw]."""
    T = M12.reshape(CCOL, P, R12)        # [c', k, m]
    out = []
    for g in range(NG):
        blk = np.zeros((P, P))
        for cp in range(CCOL):
            for mw in range(4):
                blk[4 * cp + mw, :] = T[cp, :, 4 * g + mw]
        out.append(blk)
    return out


def host_constants(target, A, x0):
    """All f64 precompute; returns the [128, BLOB_COLS] f32 device blob."""
    A64 = np.asarray(A, np.float64)
    w = ALPHA + 2 * GAMMA * (1 - np.cos(2 * np.pi * np.arange(N // 2 + 1) / N))

    def C_inv(z):
        return np.fft.irfft(np.fft.rfft(z, axis=-1) / w, n=N, axis=-1)

    U = C_inv(A64).T                              # [N, 9]
    S = np.eye(R9) + A64 @ U
    W2 = U @ np.linalg.inv(S)                     # [N, 9]
    g = np.fft.irfft(1.0 / w, n=N)                # kernel of C^-1
    b = A64 @ np.asarray(target, np.float64)
    bA = b @ A64
    c0 = C_inv(bA) - W2 @ (U.T @ bA)              # B^-1 @ bA

    blob = np.zeros((P, BLOB_COLS), np.float32)

    def putb(off, arr):
        p = _pack_bf16(arr)
        blob[:p.shape[0], off:off + p.shape[1]] = p

    mA1 = _banded({-1: GAMMA, 0: -GAMMA})         # gamma*(S+ - I)
    mB = _banded({1: GAMMA, 0: -GAMMA})           # gamma*(S- - I)
    mG = _banded({d: g[d % N] for d in range(-RB, RB + 1)})
    Ghi = [_bf16(m).astype(np.float64) for m in mG]
    Glo = [m - h for m, h in zip(mG, Ghi)]

    putb(OFF_A1M, mA1[0]); putb(OFF_A1C, mA1[1])
    putb(OFF_BM, mB[0]); putb(OFF_BC, mB[2])
    putb(OFF_ABM, mA1[0] + mB[0])
    putb(OFF_GHIM, Ghi[0]); putb(OFF_GLOM, Glo[0])
    putb(OFF_GBLH, Ghi[1])
    putb(OFF_GBHH, Ghi[2])
    putb(OFF_ONES, np.ones((P, P)))
    putb(OFF_NAI, -ALPHA * np.eye(P))

    # U2[k, m*32+c] = U[k + 128c, m] (m-major)
    putb(OFF_U2, U.reshape(CCOL, P, R9).transpose(1, 2, 0).reshape(P, R9 * CCOL))
    putb(OFF_X0, _tile(np.asarray(x0, np.float64)))

    # qS mask: M96[p, 32g + c] = (p//4 == c)
    m96 = np.zeros((P, NG * CCOL))
    for p in range(P):
        m96[p, (p // 4)::CCOL] = 1.0
    putb(OFF_M96, m96)
    putb(OFF_NIB, -np.eye(P))

    # W2e padded to 12 columns: [W2 | c0 | 0 | 0]
    W2e = np.concatenate([W2, c0[:, None], np.zeros((N, 2))], axis=1)
    AB_W2e = GAMMA * (np.roll(W2e, 1, axis=0) + np.roll(W2e, -1, axis=0)
                      - 2 * W2e)
    B_W2e = GAMMA * (np.roll(W2e, -1, axis=0) - W2e)

    for gi, blk in enumerate(_mask_blocks(-W2e)):
        putb(OFF_WS + 64 * gi, blk)
    for gi, blk in enumerate(_mask_blocks(-AB_W2e)):
        putb(OFF_ABS + 64 * gi, blk)
    for gi, blk in enumerate(_mask_blocks(-B_W2e)):
        putb(OFF_BS + 64 * gi, blk)
    for gi, blk in enumerate(_mask_blocks(ALPHA * W2e)):
        putb(OFF_CS + 64 * gi, blk)
    return np.ascontiguousarray(blob)


def build_nc():
    """Build and compile the Bacc graph (one core's program)."""
    from concourse import bacc, mybir, tile

    f32 = mybir.dt.float32
    bf16 = mybir.dt.bfloat16
    Alu = mybir.AluOpType
    ActCopy = mybir.ActivationFunctionType.Copy
    nc = bacc.Bacc(target_bir_lowering=False)

    blob_ext = nc.declare_dram_parameter("blob", [P, BLOB_COLS], f32, isOutput=False)
    out_ext = nc.declare_dram_parameter("out", [P, CCOL], f32, isOutput=True)

    with tile.TileContext(nc) as tc:
        with (
            tc.tile_pool(name="const", bufs=1) as cpool,
            tc.tile_pool(name="work", bufs=3) as wpool,
            tc.tile_pool(name="psum", bufs=1, space="PSUM") as ppool,
        ):
            cb = cpool.tile([P, BLOB_COLS], f32, tag="blob")
            nc.sync.dma_start(cb[:, :], blob_ext[:, :])

            def csb(off, wcols):
                """bf16 view of wcols f32 columns -> [128, 2*wcols] bf16"""
                return cb[:, off:off + wcols].bitcast(bf16)

            A1_M, A1_C = csb(OFF_A1M, 64), csb(OFF_A1C, 64)
            B_M, B_C = csb(OFF_BM, 64), csb(OFF_BC, 64)
            AB_M = csb(OFF_ABM, 64)
            GHI_M, GLO_M = csb(OFF_GHIM, 64), csb(OFF_GLOM, 64)
            GBL_H = csb(OFF_GBLH, 64)
            GBH_H = csb(OFF_GBHH, 64)
            ones_bf = csb(OFF_ONES, 64)
            NAI = csb(OFF_NAI, 64)
            U2 = csb(OFF_U2, 144)                  # [128, 288] bf16
            U2_3d = U2.rearrange("k (m c) -> k m c", c=CCOL)
            M96 = csb(OFF_M96, 48)                 # [128, 96] bf16
            NIB = csb(OFF_NIB, 64)                 # [128, 128] bf16 -I
            WS = [csb(OFF_WS + 64 * g, 64) for g in range(NG)]
            ABS = [csb(OFF_ABS + 64 * g, 64) for g in range(NG)]
            BS = [csb(OFF_BS + 64 * g, 64) for g in range(NG)]
            CS = [csb(OFF_CS + 64 * g, 64) for g in range(NG)]

            def corner(bank, lhsT, src, shift, stop=False, start=False):
                """Cross-column corner of a banded circulant: out col c
                reads src col c+shift (mod 32), as two payload matmuls."""
                if shift == -1:
                    nc.tensor.matmul(bank[:, 1:CCOL], lhsT, src[:, 0:CCOL - 1],
                                     start=start, stop=False,
                                     skip_group_check=True)
                    nc.tensor.matmul(bank[:, 0:1], lhsT, src[:, CCOL - 1:CCOL],
                                     start=start, stop=stop,
                                     skip_group_check=True)
                else:
                    nc.tensor.matmul(bank[:, 0:CCOL - 1], lhsT, src[:, 1:CCOL],
                                     start=start, stop=False,
                                     skip_group_check=True)
                    nc.tensor.matmul(bank[:, CCOL - 1:CCOL], lhsT, src[:, 0:1],
                                     start=start, stop=stop,
                                     skip_group_check=True)

            # persistent tiles (Z1r double-buffered to relax the WAR between
            # one iteration's reduce and the previous Qrep weight loads)
            Z1rA = cpool.tile([P, 16], bf16, tag="Z1rA")
            Z1rB = cpool.tile([P, 16], bf16, tag="Z1rB")
            for zz in (Z1rA, Z1rB):
                nc.vector.memset(zz[:, :], 0.0)
                nc.vector.memset(zz[:, R9:R9 + 1], -1.0 / P)
            bankQ = ppool.tile([P, NG * CCOL], f32, tag="Q")

            def at_bank(j):
                return ppool.tile([P, CCOL], f32, tag=f"At{j % 2}",
                                  name=f"At{j % 2}")
            def b_bank(j):
                return ppool.tile([P, CCOL], f32, tag=f"B{j % 2}",
                                  name=f"B{j % 2}")
            def cx_bank(j):
                return ppool.tile([P, CCOL], f32, tag=f"Cx{j % 2}",
                                  name=f"Cx{j % 2}")

            # --- prologue: iteration-1 banks straight from x0 ---
            x0m = csb(OFF_X0, 16)[:, 0:CCOL]
            bankCx = cx_bank(1)
            nc.tensor.matmul(bankCx[:, :], NAI, x0m, start=True, stop=True)
            bankAt = at_bank(1)
            nc.tensor.matmul(bankAt[:, :], A1_M, x0m, start=True, stop=False)
            corner(bankAt, A1_C, x0m, -1, stop=True)
            bankB = None
            T3 = None

            for j in range(1, NIT + 1):
                first = (j == 1)
                last = (j == NIT)

                # --- DVE: tau chain ---
                # u3 = tTW - (-a*x) = tau;  tTW' = min(u3, -a*x)
                t3n = wpool.tile([P, CCOL], f32, tag=f"t3{j % 2}")
                if first:
                    nc.vector.tensor_scalar_min(t3n[:, :], bankCx[:, :], 0.0)
                else:
                    u3t = wpool.tile([P, CCOL], f32, tag="u3")
                    nc.vector.tensor_sub(u3t[:, :], T3[:, :], bankCx[:, :])
                    nc.vector.tensor_tensor(t3n[:, :], u3t[:, :], bankCx[:, :],
                                            Alu.min)

                # --- DVE: soft-threshold and v (t2 = Un - eta = -tAB') ---
                r1 = wpool.tile([P, CCOL], f32, tag="r1")
                Un = wpool.tile([P, CCOL], f32, tag="Un")
                vh = wpool.tile([P, CCOL], bf16, tag="vh")
                vm = vh[:, 0:CCOL]
                nc.vector.tensor_scalar(r1[:, :], bankAt[:, :], -LAM, LAM,
                                        Alu.max, Alu.min)
                nc.vector.tensor_sub(Un[:, :], bankAt[:, :], r1[:, :])
                if first:
                    t2 = Un
                    nc.vector.tensor_sub(vm, Un[:, :], t3n[:, :])
                else:
                    t2 = wpool.tile([P, CCOL], f32, tag="t2")
                    nc.vector.tensor_sub(t2[:, :], Un[:, :], bankB[:, :])
                    nc.vector.tensor_sub(vm, t2[:, :], t3n[:, :])


                # --- PE: banded G apply on v ---
                bankE = ppool.tile([P, CCOL], f32, tag="E")
                nc.tensor.matmul(bankE[:, :], GHI_M, vm, start=True, stop=False)
                nc.tensor.matmul(bankE[:, :], GLO_M, vm, start=False, stop=False)
                corner(bankE, GBL_H, vm, -1)
                corner(bankE, GBH_H, vm, +1, stop=(not last))

                # --- DVE: rank-9 head  Z1 = U2 (.) v ; grouped c-reduce ---
                Z1r = Z1rA if j % 2 else Z1rB
                Z1 = wpool.tile([P, R9 * CCOL], bf16, tag="Z1")
                z1_3d = Z1[:, :].rearrange("k (m c) -> k m c", c=CCOL)
                vb9 = vm.unsqueeze(1).broadcast_to([P, R9, CCOL])
                nc.vector.tensor_mul(z1_3d, U2_3d, vb9)
                with nc.allow_low_precision(reason="q partials consumed f32"):
                    nc.vector.tensor_reduce(Z1r[:, 0:R9], z1_3d,
                                            axis=mybir.AxisListType.X,
                                            op=Alu.add)

                # --- DVE: replicate Z1r into matmul-ready lhsT layout ---
                # Z1rep[k, 128g + 4c' + mw] = Z1r[k, 4g + mw]
                Z1rep = wpool.tile([P, NG * P], bf16, tag="Z1rep")
                zin = Z1r[:, 0:R12].rearrange("k (g m) -> k g m", m=4) \
                    .unsqueeze(2).broadcast_to([P, NG, CCOL, 4])
                zout = Z1rep[:, :].rearrange("k (g c m) -> k g c m",
                                             c=CCOL, m=4)
                nc.vector.tensor_copy(zout, zin)

                # --- DVE: hi/lo bf16 split of the state seed t2 (PE applies
                #     -I @ (t2h + t2l) into the next At/B banks) ---
                if not last:
                    t2h = wpool.tile([P, CCOL], bf16, tag="t2h")
                    t2l = wpool.tile([P, CCOL], bf16, tag="t2l")
                    nc.vector.tensor_copy(t2h[:, :], t2[:, :])
                    nc.vector.scalar_tensor_tensor(t2l[:, :], t2[:, :],
                                                   1.0, t2h[:, :],
                                                   Alu.mult, Alu.subtract)

                # --- PE: pipeline-warming fillers (gated on Z1rep so they
                #     run back-to-back right before the Qrep matmuls) ---
                for _f in range(2):
                    nc.tensor.matmul(bankQ[0:1, 0:1], Z1rep[:, 0:1],
                                     ones_bf[:, 0:1], start=True, stop=True,
                                     skip_group_check=True)

                # --- PE: Qrep (partition reduce + replicate of q) ---
                for g in range(NG):
                    nc.tensor.matmul(bankQ[:, CCOL * g:CCOL * (g + 1)],
                                     Z1rep[:, P * g:P * (g + 1)],
                                     ones_bf[:, 0:CCOL],
                                     start=True, stop=True,
                                     skip_group_check=True)

                # --- DVE: qS = mask (.) Qrep ---
                qS = wpool.tile([P, NG * CCOL], bf16, tag="qS")
                nc.vector.tensor_mul(qS[:, :], M96[:, 0:NG * CCOL],
                                     bankQ[:, :])
                # --- Scalar: Esb cast (fully off the DVE queue) ---
                if not last:
                    Enew = wpool.tile([P, CCOL], bf16, tag="Esb")
                    nc.scalar.activation(Enew[:, :], bankE[:, :], ActCopy,
                                         bias=0.0, scale=1.0)

                # --- PE: masked rank-10 parts + plain banded bursts that
                #     COMPLETE the next iteration's banks ---
                if last:
                    for g in range(NG):
                        nc.tensor.matmul(bankE[:, :], WS[g],
                                         qS[:, CCOL * g:CCOL * (g + 1)],
                                         start=False, stop=(g == NG - 1),
                                         skip_group_check=True)
                else:
                    at_n = at_bank(j + 1)
                    b_n = b_bank(j + 1)
                    cx_n = cx_bank(j + 1)
                    em = Enew[:, 0:CCOL]
                    # plain parts first (gated only on Enew/t2h - they run
                    # during the q tail), masked parts last (gated on qS):
                    # after qS lands only 9 small matmuls separate it from
                    # the banks completing.
                    nc.tensor.matmul(at_n[:, :], AB_M, em, start=True,
                                     stop=False, skip_group_check=True)
                    corner(at_n, A1_C, em, -1)
                    corner(at_n, B_C, em, +1)
                    nc.tensor.matmul(at_n[:, :], NIB, t2h[:, :],
                                     start=False, stop=False,
                                     skip_group_check=True)
                    nc.tensor.matmul(at_n[:, :], NIB, t2l[:, :],
                                     start=False, stop=False,
                                     skip_group_check=True)
                    nc.tensor.matmul(cx_n[:, :], NAI, em, start=True,
                                     stop=False, skip_group_check=True)
                    nc.tensor.matmul(b_n[:, :], B_M, em, start=True,
                                     stop=False, skip_group_check=True)
                    corner(b_n, B_C, em, +1)
                    nc.tensor.matmul(b_n[:, :], NIB, t2h[:, :],
                                     start=False, stop=False,
                                     skip_group_check=True)
                    nc.tensor.matmul(b_n[:, :], NIB, t2l[:, :],
                                     start=False, stop=False,
                                     skip_group_check=True)
                    # masked rank-10 parts: Cx first (feeds the tau chain),
                    # then At (gates r1), then B
                    for g in range(NG):
                        nc.tensor.matmul(cx_n[:, :], CS[g],
                                         qS[:, CCOL * g:CCOL * (g + 1)],
                                         start=False, stop=(g == NG - 1),
                                         skip_group_check=True)
                    for g in range(NG):
                        nc.tensor.matmul(at_n[:, :], ABS[g],
                                         qS[:, CCOL * g:CCOL * (g + 1)],
                                         start=False, stop=(g == NG - 1),
                                         skip_group_check=True)
                    for g in range(NG):
                        nc.tensor.matmul(b_n[:, :], BS[g],
                                         qS[:, CCOL * g:CCOL * (g + 1)],
                                         start=False, stop=(g == NG - 1),
                                         skip_group_check=True)

                if not last:
                    bankAt, bankB, bankCx, T3 = at_n, b_n, cx_n, t3n
                else:
                    Xout = wpool.tile([P, CCOL], f32, tag="Xout")
                    nc.vector.tensor_copy(Xout[:, :], bankE[:, :])
                    nc.sync.dma_start(out_ext[:, :], Xout[:, :])

    nc.compile()
    return nc


def kernel(**inputs):
    from concourse.bass_utils import run_bass_kernel_spmd

    target = np.asarray(inputs["target"], np.float32)
    A = np.asarray(inputs["A"], np.float32)
    x0 = np.asarray(inputs["x0"], np.float32)

    blob = host_constants(target, A, x0)
    nc = build_nc()
    in_maps = [{"blob": blob} for _ in range(NCORES)]
    res = run_bass_kernel_spmd(nc, in_maps, core_ids=list(range(NCORES)))
    out_tile = np.asarray(res.results[0]["out"], np.float32)
    return np.ascontiguousarray(out_tile.T.reshape(-1))
